# revision 9
# baseline (speedup 1.0000x reference)
"""Trainium2 Bass kernel for masked multi-modal causal dot-product attention.

Computation (reference):
  Q = mlp(x1, Wq)               # (4096, 64), 3 linear layers, relu between
  for m in 0..3:
    K_m = mlp(x_m, Wk[m])       # (4096, 64)
    mask_m[i,j] = t2_m[j] <= t1[i]   (timestamps sorted -> staircase mask)
    acc += ((Q @ K_m.T) * mask_m) @ x_m[:, :2]
  out = acc  # (1, 4096, 2)

Sharding: 8 cores = 4 modalities x 2 query-parity halves (queries interleaved
by 128-chunks for load balance: causal visible mass grows with i).
All cores run ONE SPMD program; per-core variation is carried entirely by the
input tensors. Tile classification (fully-visible / boundary / invisible) is
computed host-side from the actual timestamps, quantified over ALL cores, so
the single program is exact for any inputs.

On-device per core:
  - MLPs in transposed layout (contraction d on partitions), f32r matmuls.
  - S^T tiles [128 j, 512 i] = K^T_tile.T @ Q^T block (f32r, full PE rate).
  - boundary tiles: mask = (t1_bcast >= t2_col) on DVE, then multiply.
  - AV: out^T[2, i] += V2_chunk.T @ S^T_tile, accumulated in PSUM over j.
Host gathers per-core [2, 2048] partials: sums over modality, interleaves
parity chunks, transposes to (1, 4096, 2).
"""

import os
import sys

import numpy as np

sys.path.insert(0, "/opt/trn_rl_repo")

T = 4096
D = 64
M = 4
NLIN = 3
NQ = 2048          # packed queries per core
CHUNK = 128        # key tile (partition dim of S^T)
IBLK = 512         # query block (moving dim)
NBLK = NQ // IBLK  # 4 query blocks per core
NJT = T // CHUNK   # 32 key tiles

LAST_RESULTS = None


def _build_program(J, F):
    """Build the SPMD Bass program.

    J[b]: number of key tiles to process for query block b (rest invisible).
    F[b]: key tiles jt < F[b] are fully visible (plain copy); F<=jt<J masked.
    """
    import concourse.bacc as bacc
    import concourse.mybir as mybir
    import concourse.tile as tile

    f32 = mybir.dt.float32
    f32r = mybir.dt.float32r
    Relu = mybir.ActivationFunctionType.Relu
    Identity = mybir.ActivationFunctionType.Identity
    is_ge = mybir.AluOpType.is_ge

    nc = bacc.Bacc("TRN2", target_bir_lowering=False, debug=False, num_devices=8)

    xqT = nc.dram_tensor("xqT", [D, NQ], f32, kind="ExternalInput")
    xkT = nc.dram_tensor("xkT", [D, T], f32, kind="ExternalInput")
    xkv = nc.dram_tensor("xkv", [CHUNK, NJT * 2], f32, kind="ExternalInput")
    xt2 = nc.dram_tensor("xt2", [CHUNK, NJT], f32, kind="ExternalInput")
    t1p = nc.dram_tensor("t1p", [1, NQ], f32, kind="ExternalInput")
    wq = nc.dram_tensor("wq", [D, NLIN * D], f32, kind="ExternalInput")
    bq = nc.dram_tensor("bq", [D, NLIN], f32, kind="ExternalInput")
    wk = nc.dram_tensor("wk", [D, NLIN * D], f32, kind="ExternalInput")
    bk = nc.dram_tensor("bk", [D, NLIN], f32, kind="ExternalInput")
    out = nc.dram_tensor("out", [2, NQ], f32, kind="ExternalOutput")

    def rr(ap):
        return ap.bitcast(f32r)

    with tile.TileContext(nc) as tc:
        with (
            tc.tile_pool(name="const", bufs=1) as const,
            tc.tile_pool(name="hq", bufs=2) as hqp,
            tc.tile_pool(name="hk", bufs=2) as hkp,
            tc.tile_pool(name="spool", bufs=4) as spool,
            tc.tile_pool(name="mpool", bufs=2) as mpool,
            tc.tile_pool(name="ps_mlp", bufs=2, space="PSUM") as ps_mlp,
            tc.tile_pool(name="ps_s", bufs=3, space="PSUM") as ps_s,
            tc.tile_pool(name="ps_o", bufs=2, space="PSUM") as ps_o,
        ):
            # ---- load constants/inputs into SBUF (chunked for overlap)
            xqT_sb = const.tile([D, NQ], f32r)
            xkT_sb = const.tile([D, T], f32r)
            for nb in range(NQ // IBLK):
                sl = slice(nb * IBLK, (nb + 1) * IBLK)
                nc.sync.dma_start(xqT_sb[:, sl], rr(xqT[:, sl]))
            for nb in range(T // IBLK):
                sl = slice(nb * IBLK, (nb + 1) * IBLK)
                nc.sync.dma_start(xkT_sb[:, sl], rr(xkT[:, sl]))
            xkv_sb = const.tile([CHUNK, NJT, 2], f32r)
            nc.sync.dma_start(xkv_sb[:], rr(xkv[:]).rearrange("p (c f) -> p c f", f=2))
            xt2_sb = const.tile([CHUNK, NJT], f32)
            nc.sync.dma_start(xt2_sb[:], xt2[:])
            t1b_sb = const.tile([CHUNK, NQ], f32)
            nc.sync.dma_start(t1b_sb[:], t1p[:].partition_broadcast(CHUNK))
            wq_sb = const.tile([D, NLIN, D], f32r)
            nc.sync.dma_start(wq_sb[:], rr(wq[:]).rearrange("p (l e) -> p l e", l=NLIN))
            bq_sb = const.tile([D, NLIN], f32)
            nc.sync.dma_start(bq_sb[:], bq[:])
            wk_sb = const.tile([D, NLIN, D], f32r)
            nc.sync.dma_start(wk_sb[:], rr(wk[:]).rearrange("p (l e) -> p l e", l=NLIN))
            bk_sb = const.tile([D, NLIN], f32)
            nc.sync.dma_start(bk_sb[:], bk[:])
            out_sb = const.tile([2, NQ], f32)

            # ---- MLPs (transposed layout: [d, t]; contraction on partitions)
            # K and Q layers are interleaved so PE never waits on one MLP's
            # ACT epilogue; epilogues alternate ACT/DVE to split the load.
            def mlp_layer(cur, w_sb, b_sb, pool, nt, layer, eng):
                nxt = pool.tile([D, nt], f32r, tag="h")
                for nb in range(nt // IBLK):
                    sl = slice(nb * IBLK, (nb + 1) * IBLK)
                    ps = ps_mlp.tile([D, IBLK], f32)
                    nc.tensor.matmul(
                        ps[:], w_sb[:, layer, :], cur[:, sl],
                        start=True, stop=True,
                    )
                    bias = b_sb[:, layer : layer + 1]
                    if eng == "act":
                        func = Relu if layer < NLIN - 1 else Identity
                        nc.scalar.activation(nxt[:, sl], ps[:], func, bias=bias)
                    else:
                        if layer < NLIN - 1:
                            nc.vector.tensor_scalar(
                                nxt[:, sl], ps[:], bias, 0.0,
                                op0=mybir.AluOpType.add, op1=mybir.AluOpType.max,
                            )
                        else:
                            nc.vector.tensor_scalar(
                                nxt[:, sl], ps[:], bias, None,
                                op0=mybir.AluOpType.add,
                            )
                return nxt

            hk, hq = xkT_sb, xqT_sb
            for layer in range(NLIN):
                hk = mlp_layer(hk, wk_sb, bk_sb, hkp, T, layer, "act")
                hq = mlp_layer(hq, wq_sb, bq_sb, hqp, NQ, layer, "dve")
            kT_sb, qT_sb = hk, hq

            # ---- main: S^T = K_tile @ Q_blk^T ; mask/copy ; out += V2^T @ S^T
            # PE stream is software-pipelined: AV(step k) is emitted after
            # QK(step k+1), so the PSUM->SBUF copy of step k overlaps the
            # next QK matmul instead of stalling PE's in-order queue.
            # Mask compares run on the otherwise-idle GpSimd engine; the
            # PSUM->SBUF traffic (copy or mask-multiply) alternates ACT/DVE.
            def emit_av(ov, s_sb, b, jt):
                nc.tensor.matmul(
                    ov[:], xkv_sb[:, jt, :], s_sb[:],
                    start=(jt == 0), stop=(jt == J[b] - 1),
                    skip_group_check=True,
                )
                if jt == J[b] - 1:
                    isl = slice(b * IBLK, (b + 1) * IBLK)
                    nc.scalar.copy(out_sb[:, isl], ov[:])

            alt = 0
            prev = None
            for b in range(NBLK):
                isl = slice(b * IBLK, (b + 1) * IBLK)
                ov = ps_o.tile([2, IBLK], f32)
                for jt in range(J[b]):
                    jsl = slice(jt * CHUNK, (jt + 1) * CHUNK)
                    sp = ps_s.tile([CHUNK, IBLK], f32)
                    nc.tensor.matmul(
                        sp[:], kT_sb[:, jsl], qT_sb[:, isl],
                        start=True, stop=True, skip_group_check=True,
                    )
                    s_sb = spool.tile([CHUNK, IBLK], f32r)
                    if jt < F[b]:
                        if alt % 2 == 0:
                            nc.scalar.copy(s_sb[:], sp[:])
                        else:
                            nc.vector.tensor_copy(s_sb[:], sp[:])
                    else:
                        mk = mpool.tile([CHUNK, IBLK], f32)
                        nc.vector.tensor_scalar(
                            mk[:], t1b_sb[:, isl], xt2_sb[:, jt : jt + 1], None,
                            op0=is_ge,
                        )
                        if alt % 2 == 0:
                            nc.scalar.copy(s_sb[:], sp[:])
                            nc.vector.tensor_mul(s_sb[:], s_sb[:], mk[:])
                        else:
                            nc.vector.tensor_mul(s_sb[:], sp[:], mk[:])
                    alt += 1
                    if prev is not None:
                        emit_av(*prev)
                    prev = (ov, s_sb, b, jt)
            emit_av(*prev)

            nc.sync.dma_start(out[:], out_sb[:])

    nc.compile()
    return nc


def kernel(x1, x2, x3, x4, Wq_w, Wq_b, Wk_w, Wk_b):
    from concourse.bass_utils import run_bass_kernel_spmd

    global LAST_RESULTS

    xs = [np.asarray(a, dtype=np.float32)[0, 0] for a in (x1, x2, x3, x4)]
    Wq_w = np.asarray(Wq_w, dtype=np.float32)
    Wq_b = np.asarray(Wq_b, dtype=np.float32)
    Wk_w = np.asarray(Wk_w, dtype=np.float32)
    Wk_b = np.asarray(Wk_b, dtype=np.float32)

    t1 = xs[0][:, -1]
    t2s = [x[:, -1] for x in xs]

    # ---- universal tile classification (exact, quantified over all cores)
    J = []
    F = []
    for b in range(NBLK):
        blk_lo = t1[1024 * b]
        blk_hi = t1[1024 * b + 1023]
        need = 0
        full = NJT
        for m in range(M):
            nvis = int(np.searchsorted(t2s[m], blk_hi, side="right"))
            nfull = int(np.searchsorted(t2s[m], blk_lo, side="right"))
            need = max(need, -(-nvis // CHUNK))
            full = min(full, nfull // CHUNK)
        J.append(max(need, 1))
        F.append(min(full, max(need, 1)))

    nc = _build_program(J, F)

    # ---- per-core host packing
    perm = np.empty((2, NQ), dtype=np.int64)
    for p in range(2):
        idx = []
        for k in range(16):
            g = 2 * k + p
            idx.append(np.arange(128 * g, 128 * g + 128))
        perm[p] = np.concatenate(idx)

    wq_h = np.ascontiguousarray(Wq_w.transpose(1, 0, 2).reshape(D, NLIN * D))
    bq_h = np.ascontiguousarray(Wq_b.T)
    x1T = np.ascontiguousarray(xs[0].T)

    in_maps = []
    for c in range(8):
        m, p = c // 2, c % 2
        xm = xs[m]
        xkv = np.ascontiguousarray(
            xm[:, 0:2].reshape(NJT, CHUNK, 2).transpose(1, 0, 2)
        ).reshape(CHUNK, NJT * 2)
        xt2 = np.ascontiguousarray(
            xm[:, D - 1].reshape(NJT, CHUNK).T
        )
        in_maps.append(
            {
                "xqT": np.ascontiguousarray(x1T[:, perm[p]]),
                "xkT": np.ascontiguousarray(xm.T),
                "xkv": xkv,
                "xt2": xt2,
                "t1p": np.ascontiguousarray(t1[perm[p]][None, :]),
                "wq": wq_h,
                "bq": bq_h,
                "wk": np.ascontiguousarray(
                    Wk_w[m].transpose(1, 0, 2).reshape(D, NLIN * D)
                ),
                "bk": np.ascontiguousarray(Wk_b[m].T),
            }
        )

    res = run_bass_kernel_spmd(nc, in_maps, core_ids=list(range(8)))
    LAST_RESULTS = res

    # ---- gather: sum over modalities, unpermute parity chunks, transpose
    acc = np.zeros((2, T), dtype=np.float32)
    for c in range(8):
        m, p = c // 2, c % 2
        acc[:, perm[p]] += res.results[c]["out"]
    return np.ascontiguousarray(acc.T)[None]
